# revision 1
# baseline (speedup 1.0000x reference)
"""GQA attention block on 8 trn2 NeuronCores.

Sharding: core c = (batch b=c//4, kv-head-pair g=c%4). Each core owns kv heads
{2g, 2g+1} and their 8 query heads (GQA tile mapping: q-head i -> kv-head i%8),
with Wq/Wk/Wv column-sharded and Wo row-sharded; host sums the 4 partial
outputs per batch and adds bo.

Device layout strategy (per core):
  - host stages q^T/k^T/v^T (bf16) so every matmul has its contraction dim on
    partitions with no device-side transposes.
  - RoPE folded into doubled projection weights Wt=[W | rot(W)] (host permuted)
    + elementwise cos/sin combine on DVE.
  - attention computed transposed: scores^T[k,q] = Kp^T(chunk)ᵀ·Qp^T, exp on
    ACT (scale=1/8 folded in, no max subtraction -- scores bounded ~|6|),
    AV via lhsT=Vp with an appended ones column giving the softmax denominator
    for free; normalization via reciprocal + K=1 outer-product broadcast.
  - out^T feeds the final projection as lhsT directly; partial [S,D] fp32 out.
"""

import os
from contextlib import ExitStack

import numpy as np
import ml_dtypes

D = 2048
QH = 32
KVH = 8
HD = 64
B = 2
S = 2048
THETA = 1000000.0
P = 128
NCORES = 8

BF16 = ml_dtypes.bfloat16

_CACHE = {}


def _build_program():
    import concourse.bass as bass
    import concourse.tile as tile
    from concourse import bacc, mybir

    nc = bacc.Bacc(
        "TRN2",
        target_bir_lowering=False,
        debug=False,
        enable_asserts=False,
        num_devices=NCORES,
    )
    bf = mybir.dt.bfloat16
    f32 = mybir.dt.float32

    qT = nc.dram_tensor("qT", [D, S], bf, kind="ExternalInput").ap()
    kT = nc.dram_tensor("kT", [D, S], bf, kind="ExternalInput").ap()
    vT = nc.dram_tensor("vT", [D, S], bf, kind="ExternalInput").ap()
    wqt = nc.dram_tensor("wqt", [D, 1024], bf, kind="ExternalInput").ap()
    wkt = nc.dram_tensor("wkt", [D, 256], bf, kind="ExternalInput").ap()
    wv = nc.dram_tensor("wv", [D, 128], bf, kind="ExternalInput").ap()
    wo = nc.dram_tensor("wo", [512, D], bf, kind="ExternalInput").ap()
    cosr = nc.dram_tensor("cosr", [P, S], f32, kind="ExternalInput").ap()
    sinr = nc.dram_tensor("sinr", [P, S], f32, kind="ExternalInput").ap()
    out = nc.dram_tensor("out", [S, D], f32, kind="ExternalOutput").ap()

    # partitioned DRAM views
    qT3 = qT.rearrange("(o p) s -> p o s", p=P)    # [128, 16, 2048]
    kT3 = kT.rearrange("(o p) s -> p o s", p=P)
    vT3 = vT.rearrange("(o p) s -> p o s", p=P)
    wqt3 = wqt.rearrange("(o p) m -> p o m", p=P)  # [128, 16, 1024]
    wkt3 = wkt.rearrange("(o p) m -> p o m", p=P)  # [128, 16, 256]
    wv3 = wv.rearrange("(o p) m -> p o m", p=P)    # [128, 16, 128]
    wo3 = wo.rearrange("(o p) d -> p o d", p=P)    # [128, 4, 2048]
    out3 = out.rearrange("(t p) d -> p t d", p=P)  # [128, 16, 2048]

    with tile.TileContext(nc) as tc, ExitStack() as ctx:
        const = ctx.enter_context(tc.tile_pool(name="const", bufs=1))
        persist = ctx.enter_context(tc.tile_pool(name="persist", bufs=1))

        # ---- resident weights / tables ----
        wqt_sb = const.tile([P, 16, 1024], bf, tag="wqt")
        nc.sync.dma_start(wqt_sb[:], wqt3[:])
        wkt_sb = const.tile([P, 16, 256], bf, tag="wkt")
        nc.sync.dma_start(wkt_sb[:], wkt3[:])
        wv_sb = const.tile([P, 16, 128], bf, tag="wv")
        nc.sync.dma_start(wv_sb[:], wv3[:])
        wo_sb = const.tile([P, 4, 2048], bf, tag="wo")
        nc.sync.dma_start(wo_sb[:], wo3[:])
        cos_sb = const.tile([P, S], f32, tag="cos")
        nc.sync.dma_start(cos_sb[:], cosr[:])
        sin_sb = const.tile([P, S], f32, tag="sin")
        nc.sync.dma_start(sin_sb[:], sinr[:])
        ones_sb = const.tile([1, 64], f32, tag="ones")
        nc.vector.memset(ones_sb[:], 1.0)

        # ---- persistent intermediates ----
        kpt_b = persist.tile([64, 2, S], bf, tag="kpt")      # rotated K^T per kv head
        qpt_b = persist.tile([64, 8, 2048], bf, tag="qpt")   # rotated Q^T per q head
        vp_sb = persist.tile([P, 16, 130], bf, tag="vp")     # Vp + ones cols
        outT_b = persist.tile([P, 4, 2048], bf, tag="outT")  # unnormalized out^T
        nc.vector.memset(vp_sb[:, :, 64:65], 1.0)
        nc.vector.memset(vp_sb[:, :, 129:130], 1.0)

        # =============== Phase 1-3: projections ===============
        with ExitStack() as pctx:
            bigin = pctx.enter_context(tc.tile_pool(name="bigin", bufs=1))
            kstream = pctx.enter_context(tc.tile_pool(name="kstream", bufs=4))
            ptmp = pctx.enter_context(tc.tile_pool(name="ptmp", bufs=2))
            ppsum = pctx.enter_context(
                tc.tile_pool(name="ppsum", bufs=4, space="PSUM")
            )

            # ---- V projection: direct Vp [s,128] via lhsT = vT slices ----
            for quarter in range(4):
                vh_sb = bigin.tile([P, 16, 512], bf, tag="bigin")
                for o in range(16):
                    nc.sync.dma_start(
                        vh_sb[:, o, :], vT3[:, o, quarter * 512 : (quarter + 1) * 512]
                    )
                for st in range(4):  # s-tiles of 128 within this quarter
                    psv_full = ppsum.tile([P, 512], f32, tag="pp", name="psv")
                    psv = psv_full[:, :128]
                    for o in range(16):
                        nc.tensor.matmul(
                            psv,
                            lhsT=vh_sb[:, o, st * 128 : (st + 1) * 128],
                            rhs=wv_sb[:, o, :],
                            start=(o == 0),
                            stop=(o == 15),
                        )
                    kt_idx = quarter * 4 + st
                    nc.vector.tensor_copy(out=vp_sb[:, kt_idx, 0:64], in_=psv[:, 0:64])
                    nc.vector.tensor_copy(
                        out=vp_sb[:, kt_idx, 65:129], in_=psv[:, 64:128]
                    )

            # ---- K projection + RoPE: KpT_rot per kv head ----
            for ns in range(4):
                ps_kp = ppsum.tile([P, 512], f32, tag="pp")
                ps_kr = ppsum.tile([P, 512], f32, tag="pp")
                for o in range(16):
                    ktile = kstream.tile([P, 512], bf, tag="kt")
                    nc.sync.dma_start(
                        ktile[:], kT3[:, o, ns * 512 : (ns + 1) * 512]
                    )
                    nc.tensor.matmul(
                        ps_kp,
                        lhsT=wkt_sb[:, o, 0:128],
                        rhs=ktile[:],
                        start=(o == 0),
                        stop=(o == 15),
                    )
                    nc.tensor.matmul(
                        ps_kr,
                        lhsT=wkt_sb[:, o, 128:256],
                        rhs=ktile[:],
                        start=(o == 0),
                        stop=(o == 15),
                    )
                sl = slice(ns * 512, (ns + 1) * 512)
                t1 = ptmp.tile([P, 512], f32, tag="t1")
                t2 = ptmp.tile([P, 512], f32, tag="t2")
                nc.vector.tensor_mul(out=t1[:], in0=ps_kp[:], in1=cos_sb[:, sl])
                nc.vector.tensor_mul(out=t2[:], in0=ps_kr[:], in1=sin_sb[:, sl])
                for lh in range(2):
                    lp = slice(lh * 64, lh * 64 + 64)
                    nc.vector.tensor_add(
                        out=kpt_b[:, lh, sl], in0=t1[lp, :], in1=t2[lp, :]
                    )

        # ======= unified pipeline: per s-quarter Qproj -> attn -> outproj =======
        with ExitStack() as mctx:
            bigin = mctx.enter_context(tc.tile_pool(name="bigin2", bufs=1))
            ptmp = mctx.enter_context(tc.tile_pool(name="ptmp2", bufs=2))
            mpsum = mctx.enter_context(
                tc.tile_pool(name="mpsum", bufs=3, space="PSUM")
            )
            apsum = mctx.enter_context(
                tc.tile_pool(name="apsum", bufs=3, space="PSUM")
            )
            opsum = mctx.enter_context(
                tc.tile_pool(name="opsum", bufs=2, space="PSUM")
            )
            epool = mctx.enter_context(tc.tile_pool(name="et", bufs=24))
            ntmp = mctx.enter_context(tc.tile_pool(name="ntmp", bufs=3))
            fout = mctx.enter_context(tc.tile_pool(name="fout", bufs=3))
            scale = 1.0 / float(np.sqrt(HD))
            Exp = mybir.ActivationFunctionType.Exp

            for quarter in range(4):
                # ---- Q projection + RoPE for this s-quarter ----
                qh_sb = bigin.tile([P, 16, 512], bf, tag="bigin")
                for o in range(16):
                    nc.sync.dma_start(
                        qh_sb[:, o, :], qT3[:, o, quarter * 512 : (quarter + 1) * 512]
                    )
                for m in range(4):
                    ps_qp = mpsum.tile([P, 512], f32, tag="pp")
                    for o in range(16):
                        nc.tensor.matmul(
                            ps_qp,
                            lhsT=wqt_sb[:, o, m * 128 : (m + 1) * 128],
                            rhs=qh_sb[:, o, :],
                            start=(o == 0),
                            stop=(o == 15),
                        )
                    gs = slice(quarter * 512, (quarter + 1) * 512)
                    # rotate_half via 32-aligned partition-shifted DVE copies
                    rot = ptmp.tile([P, 512], f32, tag="rot")
                    for hh in range(2):
                        b0 = hh * 64
                        nc.vector.tensor_scalar_mul(
                            rot[b0 : b0 + 32, :], ps_qp[b0 + 32 : b0 + 64, :], -1.0
                        )
                        nc.vector.tensor_copy(
                            out=rot[b0 + 32 : b0 + 64, :], in_=ps_qp[b0 : b0 + 32, :]
                        )
                    t1 = ptmp.tile([P, 512], f32, tag="t1")
                    t2 = ptmp.tile([P, 512], f32, tag="t2")
                    nc.vector.tensor_mul(out=t1[:], in0=ps_qp[:], in1=cos_sb[:, gs])
                    nc.vector.tensor_mul(out=t2[:], in0=rot[:], in1=sin_sb[:, gs])
                    for sub in range(2):
                        lp = slice(sub * 64, sub * 64 + 64)
                        nc.vector.tensor_add(
                            out=qpt_b[:, 2 * m + sub, gs],
                            in0=t1[lp, :],
                            in1=t2[lp, :],
                        )

                # ---- attention for sc = quarter ----
                for lh in range(2):
                    for j in range(4):
                        h = lh * 4 + j
                        hp = slice((h % 2) * 64, (h % 2) * 64 + 64)
                        hc = h // 2
                        ssl = slice(quarter * 512, (quarter + 1) * 512)
                        pso = opsum.tile([65, 512], f32, tag="po")
                        for kt in range(16):
                            pss = apsum.tile([P, 512], f32, tag="ps")
                            nc.tensor.matmul(
                                pss,
                                lhsT=kpt_b[:, lh, kt * 128 : (kt + 1) * 128],
                                rhs=qpt_b[:, h, ssl],
                                start=True,
                                stop=True,
                            )
                            et = epool.tile([P, 512], bf, tag="et", name=f"et{kt}")
                            nc.scalar.activation(
                                out=et[:], in_=pss[:], func=Exp, scale=scale
                            )
                            nc.tensor.matmul(
                                pso,
                                lhsT=vp_sb[:, kt, lh * 65 : (lh + 1) * 65],
                                rhs=et[:],
                                start=(kt == 0),
                                stop=(kt == 15),
                            )
                        recip = ntmp.tile([1, 512], f32, tag="recip")
                        nc.vector.reciprocal(recip[:], pso[64:65, :])
                        bc = ntmp.tile([64, 512], f32, tag="bc")
                        nc.gpsimd.partition_broadcast(bc[:], recip[:])
                        nc.vector.tensor_mul(
                            out=outT_b[hp, hc, ssl], in0=pso[0:64, :], in1=bc[:]
                        )

                # ---- output projection for this quarter's s-tiles ----
                for qi in range(4):
                    qt = quarter * 4 + qi
                    for dn in range(4):
                        psf = mpsum.tile([P, 512], f32, tag="pp", name="psf")
                        for cc in range(4):
                            nc.tensor.matmul(
                                psf,
                                lhsT=outT_b[:, cc, qt * 128 : (qt + 1) * 128],
                                rhs=wo_sb[:, cc, dn * 512 : (dn + 1) * 512],
                                start=(cc == 0),
                                stop=(cc == 3),
                            )
                        of = fout.tile([P, 512], f32, tag="of")
                        nc.any.tensor_copy(out=of[:], in_=psf[:])
                        nc.sync.dma_start(
                            out3[:, qt, dn * 512 : (dn + 1) * 512], of[:]
                        )

    nc.finalize()
    return nc


def _rot_cols(W):
    """(x @ rot_cols(W)) == rotate_half(x @ W), per 64-wide head block."""
    Wr = np.empty_like(W)
    n = W.shape[1] // HD
    for h in range(n):
        blk = W[:, h * HD : (h + 1) * HD]
        Wr[:, h * HD : h * HD + 32] = -blk[:, 32:64]
        Wr[:, h * HD + 32 : h * HD + 64] = blk[:, 0:32]
    return Wr


def _host_inputs(q, k, v, Wq, Wk, Wv, Wo):
    """Build the 8 per-core input dicts."""
    inv_freq = 1.0 / (THETA ** (np.arange(0, HD, 2, dtype=np.float32) / HD))
    t = np.arange(S, dtype=np.float32)
    freqs = np.einsum("i,j->ij", t, inv_freq)
    emb = np.concatenate([freqs, freqs], axis=-1)  # [S, 64]
    cosT = np.ascontiguousarray(np.cos(emb).T, dtype=np.float32)  # [64, S]
    sinT = np.ascontiguousarray(np.sin(emb).T, dtype=np.float32)
    cos_rep = np.concatenate([cosT, cosT], axis=0)  # [128, S]
    sin_rep = np.concatenate([sinT, sinT], axis=0)

    qT = [np.ascontiguousarray(q[b].T).astype(BF16) for b in range(B)]
    kTt = [np.ascontiguousarray(k[b].T).astype(BF16) for b in range(B)]
    vTt = [np.ascontiguousarray(v[b].T).astype(BF16) for b in range(B)]

    in_maps = []
    for c in range(NCORES):
        b, g = divmod(c, 4)
        qheads = [2 * g, 2 * g + 8, 2 * g + 16, 2 * g + 24,
                  2 * g + 1, 2 * g + 9, 2 * g + 17, 2 * g + 25]
        qcols = np.concatenate([np.arange(h * HD, (h + 1) * HD) for h in qheads])
        kvcols = np.arange(2 * g * HD, (2 * g + 2) * HD)

        wq_c = np.ascontiguousarray(Wq[:, qcols])
        wqt_np = np.concatenate([wq_c, _rot_cols(wq_c)], axis=1).astype(BF16)
        wk_c = np.ascontiguousarray(Wk[:, kvcols])
        wkt_np = np.concatenate([wk_c, _rot_cols(wk_c)], axis=1).astype(BF16)
        wv_np = np.ascontiguousarray(Wv[:, kvcols]).astype(BF16)
        wo_np = np.ascontiguousarray(Wo[qcols, :]).astype(BF16)

        in_maps.append({
            "qT": qT[b], "kT": kTt[b], "vT": vTt[b],
            "wqt": wqt_np, "wkt": wkt_np, "wv": wv_np, "wo": wo_np,
            "cosr": cos_rep, "sinr": sin_rep,
        })
    return in_maps


def kernel(q, k, v, attn_mask, Wq, Wk, Wv, Wo, bo):
    from concourse.bass_utils import run_bass_kernel_spmd

    q = np.asarray(q, dtype=np.float32)
    k = np.asarray(k, dtype=np.float32)
    v = np.asarray(v, dtype=np.float32)
    Wq = np.asarray(Wq, dtype=np.float32)
    Wk = np.asarray(Wk, dtype=np.float32)
    Wv = np.asarray(Wv, dtype=np.float32)
    Wo = np.asarray(Wo, dtype=np.float32)
    bo = np.asarray(bo, dtype=np.float32)

    if "nc" not in _CACHE:
        _CACHE["nc"] = _build_program()
    nc = _CACHE["nc"]

    in_maps = _host_inputs(q, k, v, Wq, Wk, Wv, Wo)
    trace = bool(int(os.environ.get("KERNEL_TRACE", "0")))
    res = run_bass_kernel_spmd(nc, in_maps, core_ids=list(range(NCORES)),
                               trace=trace)
    _CACHE["last_result"] = res

    out = np.zeros((B, S, D), dtype=np.float32)
    for c in range(NCORES):
        b = c // 4
        out[b] += np.asarray(res.results[c]["out"], dtype=np.float32)
    out += bo[None, None, :]
    return out



# revision 5
# speedup vs baseline: 1.8229x; 1.8229x over previous
"""GQA attention block on 8 trn2 NeuronCores.

Sharding: core c = (batch b=c//4, kv-head-pair g=c%4). Each core owns kv heads
{2g, 2g+1} and their 8 query heads, with Wq/Wk/Wv column-sharded and Wo
row-sharded; host sums the 4 partial outputs per batch and adds bo.

Per-core schedule (v2 — PE-dense / ACT-dense pipeline):
  - scores row-packed: the two kv groups live on partitions 0-63 / 64-127 of
    K^T and Q^T, so each kt step issues two concurrent K=64 matmuls on
    disjoint PE row-groups into two PSUM banks (full 128-row utilization).
  - one [128,1024] exp over both banks per kt (amortizes ACT ramp overhead).
  - AV via lhsT=Vp with an appended ones column (M=65) giving the softmax
    denominator in row 64; normalization uses reciprocal_approx_fast +
    gpsimd broadcast, buffered 3-deep so it never stalls the kt pipeline.
  - RoPE: rotate_half built on DVE from the projection PSUM (no doubled
    weights), cos/sin combine writes bf16 Q^T/K^T pair tiles.
  - Q-proj / attention / O-proj share one pool scope; qpt and outT are
    double-buffered per s-quarter so quarter q+1's projections overlap
    quarter q's ACT-bound attention.
  - inputs host-restaged so every load/store is one dma_start with
    16KB/partition contiguous runs.
"""

import os
from contextlib import ExitStack

import numpy as np
import ml_dtypes

D = 2048
QH = 32
KVH = 8
HD = 64
B = 2
S = 2048
THETA = 1000000.0
P = 128
NCORES = 8

BF16 = ml_dtypes.bfloat16

_CACHE = {}


def _build_program():
    import concourse.bass as bass
    import concourse.tile as tile
    from concourse import bacc, mybir

    nc = bacc.Bacc(
        "TRN2",
        target_bir_lowering=False,
        debug=False,
        enable_asserts=False,
        num_devices=NCORES,
    )
    bf = mybir.dt.bfloat16
    f32 = mybir.dt.float32
    Exp = mybir.ActivationFunctionType.Exp

    # host-staged layouts: [p, t(=s quarter), o(=D/128 chunk), s] contiguous
    qS = nc.dram_tensor("qS", [P, 4 * 16 * 512], bf, kind="ExternalInput").ap()
    kS = nc.dram_tensor("kS", [P, 4 * 16 * 512], bf, kind="ExternalInput").ap()
    vS = nc.dram_tensor("vS", [P, 4 * 16 * 512], bf, kind="ExternalInput").ap()
    wqS = nc.dram_tensor("wqS", [P, 16 * 512], bf, kind="ExternalInput").ap()
    wkS = nc.dram_tensor("wkS", [P, 16 * 128], bf, kind="ExternalInput").ap()
    wvS = nc.dram_tensor("wvS", [P, 16 * 128], bf, kind="ExternalInput").ap()
    woS = nc.dram_tensor("woS", [P, 4 * 2048], bf, kind="ExternalInput").ap()
    cosr = nc.dram_tensor("cosr", [P, S], f32, kind="ExternalInput").ap()
    sinr = nc.dram_tensor("sinr", [P, S], f32, kind="ExternalInput").ap()
    outS = nc.dram_tensor("outS", [P, 16 * 2048], f32, kind="ExternalOutput").ap()

    q4 = qS.rearrange("p (t o s) -> p t o s", t=4, o=16, s=512)
    k4 = kS.rearrange("p (t o s) -> p t o s", t=4, o=16, s=512)
    v4 = vS.rearrange("p (t o s) -> p t o s", t=4, o=16, s=512)
    wq3 = wqS.rearrange("p (o m) -> p o m", o=16, m=512)
    wk3 = wkS.rearrange("p (o m) -> p o m", o=16, m=128)
    wv3 = wvS.rearrange("p (o m) -> p o m", o=16, m=128)
    wo3 = woS.rearrange("p (c d) -> p c d", c=4, d=2048)
    out3 = outS.rearrange("p (t d) -> p t d", t=16, d=2048)

    scale = 1.0 / float(np.sqrt(HD))

    with tile.TileContext(nc) as tc, ExitStack() as ctx:
        const = ctx.enter_context(tc.tile_pool(name="const", bufs=1))
        persist = ctx.enter_context(tc.tile_pool(name="persist", bufs=1))
        xin = ctx.enter_context(tc.tile_pool(name="xin", bufs=2))
        qptp = ctx.enter_context(tc.tile_pool(name="qptp", bufs=2))
        otp = ctx.enter_context(tc.tile_pool(name="otp", bufs=2))
        etp = ctx.enter_context(tc.tile_pool(name="etp", bufs=3))
        rtmp = ctx.enter_context(tc.tile_pool(name="rtmp", bufs=3))
        rcp = ctx.enter_context(tc.tile_pool(name="rcp", bufs=2))
        bcp = ctx.enter_context(tc.tile_pool(name="bcp", bufs=2))
        fout = ctx.enter_context(tc.tile_pool(name="fout", bufs=2))
        qpp = ctx.enter_context(tc.tile_pool(name="qpp", bufs=1, space="PSUM"))
        opp = ctx.enter_context(tc.tile_pool(name="opp", bufs=1, space="PSUM"))
        scp = ctx.enter_context(tc.tile_pool(name="scp", bufs=2, space="PSUM"))
        psop = ctx.enter_context(tc.tile_pool(name="psop", bufs=2, space="PSUM"))

        # ---- resident weights / tables ----
        wqt_sb = const.tile([P, 16, 512], bf, tag="wqt")
        nc.sync.dma_start(wqt_sb[:], wq3[:])
        wkt_sb = const.tile([P, 16, 128], bf, tag="wkt")
        nc.sync.dma_start(wkt_sb[:], wk3[:])
        wv_sb = const.tile([P, 16, 128], bf, tag="wv")
        nc.sync.dma_start(wv_sb[:], wv3[:])
        wo_sb = const.tile([P, 4, 2048], bf, tag="wo")
        nc.sync.dma_start(wo_sb[:], wo3[:])
        cos_sb = const.tile([P, S], f32, tag="cos")
        nc.sync.dma_start(cos_sb[:], cosr[:])
        sin_sb = const.tile([P, S], f32, tag="sin")
        nc.sync.dma_start(sin_sb[:], sinr[:])

        # ---- persistent intermediates ----
        kpt = persist.tile([P, S], bf, tag="kpt")      # rotated K^T, kv0|kv1 halves
        vp_t = []
        for vt in range(4):
            vv = persist.tile([P, 4, 130], bf, tag=f"vp{vt}")
            nc.vector.memset(vv[:, :, 64:65], 1.0)
            nc.vector.memset(vv[:, :, 129:130], 1.0)
            vp_t.append(vv)

        def rope(ps, dsts, tsl):
            """RoPE combine from psum tile ps [128,512] (2 head-blocks of 64)
            into dsts = [(dst_ap_for_rows_0_63), (dst_ap_for_rows_64_127)]."""
            rot = rtmp.tile([P, 512], f32, tag="rot")
            for b0 in (0, 64):
                nc.vector.tensor_scalar_mul(
                    rot[b0 : b0 + 32, :], ps[b0 + 32 : b0 + 64, :], -1.0
                )
                nc.vector.tensor_copy(
                    out=rot[b0 + 32 : b0 + 64, :], in_=ps[b0 : b0 + 32, :]
                )
            t1 = rtmp.tile([P, 512], f32, tag="t1")
            t2 = rtmp.tile([P, 512], f32, tag="t2")
            nc.vector.tensor_mul(out=t1[:], in0=ps[:], in1=cos_sb[:, tsl])
            nc.vector.tensor_mul(out=t2[:], in0=rot[:], in1=sin_sb[:, tsl])
            for i, dst in enumerate(dsts):
                rows = slice(i * 64, i * 64 + 64)
                nc.vector.tensor_add(out=dst, in0=t1[rows, :], in1=t2[rows, :])

        # ================= K projection + RoPE =================
        for t in range(4):
            kh = xin.tile([P, 16, 512], bf, tag="xin")
            nc.sync.dma_start(kh[:], k4[:, t])
            tsl = slice(t * 512, (t + 1) * 512)
            ps_full = scp.tile([P, 1024], f32, tag="sc", name="kproj")
            ps = ps_full[:, 0:512]
            for o in range(16):
                nc.tensor.matmul(
                    ps,
                    lhsT=wkt_sb[:, o, :],
                    rhs=kh[:, o, :],
                    start=(o == 0),
                    stop=(o == 15),
                )
            # kv0 -> kpt rows 0:64, kv1 -> rows 64:128 (same-row combine)
            rope(ps, [kpt[0:64, tsl], kpt[64:128, tsl]], tsl)

        # ================= V projection =================
        for t in range(4):
            vh = xin.tile([P, 16, 512], bf, tag="xin")
            nc.sync.dma_start(vh[:], v4[:, t])
            for st in range(4):
                psv = psop.tile([P, 128], f32, tag="pso", name="psv")
                for o in range(16):
                    nc.tensor.matmul(
                        psv,
                        lhsT=vh[:, o, st * 128 : (st + 1) * 128],
                        rhs=wv_sb[:, o, :],
                        start=(o == 0),
                        stop=(o == 15),
                    )
                nc.vector.tensor_copy(out=vp_t[t][:, st, 0:64], in_=psv[:, 0:64])
                nc.vector.tensor_copy(out=vp_t[t][:, st, 65:129], in_=psv[:, 64:128])

        # ========== main loop: per s-quarter Qproj -> attn -> outproj ==========
        for t in range(4):
            tsl = slice(t * 512, (t + 1) * 512)
            qh = xin.tile([P, 16, 512], bf, tag="xin")
            nc.sync.dma_start(qh[:], q4[:, t])

            # ---- Q projection + RoPE: pairs (j, j+4) on partition halves ----
            qpt = qptp.tile([P, 4, 512], bf, tag="qpt")
            for m in range(4):
                ps = qpp.tile([P, 512], f32, tag="qp", name="qproj")
                for o in range(16):
                    nc.tensor.matmul(
                        ps,
                        lhsT=wqt_sb[:, o, m * 128 : (m + 1) * 128],
                        rhs=qh[:, o, :],
                        start=(o == 0),
                        stop=(o == 15),
                    )
                dsts = []
                for h in (2 * m, 2 * m + 1):
                    base = 0 if h < 4 else 64
                    dsts.append(qpt[base : base + 64, h % 4, :])
                rope(ps, dsts, tsl)

            # ---- attention: 4 head-pairs, row-packed scores ----
            outT = otp.tile([P, 4, 512], bf, tag="outT")
            for j in range(4):
                pso0 = psop.tile([65, 512], f32, tag="pso", name="pso0")
                pso1 = psop.tile([65, 512], f32, tag="pso", name="pso1")
                for kt in range(16):
                    ksl = slice(kt * 128, (kt + 1) * 128)
                    sc = scp.tile([P, 1024], f32, tag="sc", name="sc")
                    nc.tensor.matmul(
                        sc[:, 0:512],
                        lhsT=kpt[0:64, ksl],
                        rhs=qpt[0:64, j, :],
                        start=True,
                        stop=True,
                    )
                    nc.tensor.matmul(
                        sc[:, 512:1024],
                        lhsT=kpt[64:128, ksl],
                        rhs=qpt[64:128, j, :],
                        start=True,
                        stop=True,
                    )
                    et = etp.tile([P, 1024], bf, tag="et")
                    nc.scalar.activation(out=et[:], in_=sc[:], func=Exp, scale=scale)
                    vps = vp_t[kt // 4]
                    nc.tensor.matmul(
                        pso0,
                        lhsT=vps[:, kt % 4, 0:65],
                        rhs=et[:, 0:512],
                        start=(kt == 0),
                        stop=(kt == 15),
                    )
                    nc.tensor.matmul(
                        pso1,
                        lhsT=vps[:, kt % 4, 65:130],
                        rhs=et[:, 512:1024],
                        start=(kt == 0),
                        stop=(kt == 15),
                    )
                for pp, h in ((pso0, j), (pso1, j + 4)):
                    den = rcp.tile([1, 512], f32, tag="den")
                    nc.vector.tensor_copy(out=den[:], in_=pp[64:65, :])
                    rc = rcp.tile([1, 512], f32, tag="rc")
                    nc.vector.reciprocal_approx_fast(out=rc[:], in_=den[:])
                    bc = bcp.tile([64, 512], f32, tag="bc")
                    nc.gpsimd.partition_broadcast(bc[:], rc[:])
                    hp = (h % 2) * 64
                    nc.vector.tensor_mul(
                        out=outT[hp : hp + 64, h // 2, :], in0=pp[0:64, :], in1=bc[:]
                    )

            # ---- output projection for this quarter's 4 s-tiles ----
            for qi in range(4):
                qt = t * 4 + qi
                fo = fout.tile([P, 2048], f32, tag="fo")
                for dn in range(4):
                    psf = opp.tile([P, 512], f32, tag="op", name="psf")
                    for cc in range(4):
                        nc.tensor.matmul(
                            psf,
                            lhsT=outT[:, cc, qi * 128 : (qi + 1) * 128],
                            rhs=wo_sb[:, cc, dn * 512 : (dn + 1) * 512],
                            start=(cc == 0),
                            stop=(cc == 3),
                        )
                    nc.vector.tensor_copy(
                        out=fo[:, dn * 512 : (dn + 1) * 512], in_=psf[:]
                    )
                nc.sync.dma_start(out3[:, qt], fo[:])

    nc.finalize()
    return nc


def _host_inputs(q, k, v, Wq, Wk, Wv, Wo):
    """Build the 8 per-core input dicts (host-restaged layouts)."""
    inv_freq = 1.0 / (THETA ** (np.arange(0, HD, 2, dtype=np.float32) / HD))
    tt = np.arange(S, dtype=np.float32)
    freqs = np.einsum("i,j->ij", tt, inv_freq)
    emb = np.concatenate([freqs, freqs], axis=-1)  # [S, 64]
    cosT = np.ascontiguousarray(np.cos(emb).T, dtype=np.float32)  # [64, S]
    sinT = np.ascontiguousarray(np.sin(emb).T, dtype=np.float32)
    cos_rep = np.ascontiguousarray(np.concatenate([cosT, cosT], axis=0))  # [128,S]
    sin_rep = np.ascontiguousarray(np.concatenate([sinT, sinT], axis=0))

    def stage_x(x):  # x [S, D] -> [p, t, o, s] flattened [128, 32768]
        a = np.ascontiguousarray(x.T).astype(BF16)  # [D, S]
        a = a.reshape(16, 128, 4, 512).transpose(1, 2, 0, 3)
        return np.ascontiguousarray(a).reshape(128, -1)

    xq = [stage_x(q[b]) for b in range(B)]
    xk = [stage_x(k[b]) for b in range(B)]
    xv = [stage_x(v[b]) for b in range(B)]

    in_maps = []
    for c in range(NCORES):
        b, g = divmod(c, 4)
        qheads = [2 * g, 2 * g + 8, 2 * g + 16, 2 * g + 24,
                  2 * g + 1, 2 * g + 9, 2 * g + 17, 2 * g + 25]
        qcols = np.concatenate([np.arange(h * HD, (h + 1) * HD) for h in qheads])
        kvcols = np.arange(2 * g * HD, (2 * g + 2) * HD)

        def stage_w(w, ncol):  # [D, ncol] -> [p, o, m] flattened
            a = w.astype(BF16).reshape(16, 128, ncol).transpose(1, 0, 2)
            return np.ascontiguousarray(a).reshape(128, -1)

        wq_np = stage_w(np.ascontiguousarray(Wq[:, qcols]), 512)
        wk_np = stage_w(np.ascontiguousarray(Wk[:, kvcols]), 128)
        wv_np = stage_w(np.ascontiguousarray(Wv[:, kvcols]), 128)
        wo_c = np.ascontiguousarray(Wo[qcols, :]).astype(BF16)  # [512, 2048]
        wo_np = np.ascontiguousarray(
            wo_c.reshape(4, 128, 2048).transpose(1, 0, 2)
        ).reshape(128, -1)

        in_maps.append({
            "qS": xq[b], "kS": xk[b], "vS": xv[b],
            "wqS": wq_np, "wkS": wk_np, "wvS": wv_np, "woS": wo_np,
            "cosr": cos_rep, "sinr": sin_rep,
        })
    return in_maps


def kernel(q, k, v, attn_mask, Wq, Wk, Wv, Wo, bo):
    from concourse.bass_utils import run_bass_kernel_spmd

    q = np.asarray(q, dtype=np.float32)
    k = np.asarray(k, dtype=np.float32)
    v = np.asarray(v, dtype=np.float32)
    Wq = np.asarray(Wq, dtype=np.float32)
    Wk = np.asarray(Wk, dtype=np.float32)
    Wv = np.asarray(Wv, dtype=np.float32)
    Wo = np.asarray(Wo, dtype=np.float32)
    bo = np.asarray(bo, dtype=np.float32)

    if "nc" not in _CACHE:
        _CACHE["nc"] = _build_program()
    nc = _CACHE["nc"]

    in_maps = _host_inputs(q, k, v, Wq, Wk, Wv, Wo)
    trace = bool(int(os.environ.get("KERNEL_TRACE", "0")))
    res = run_bass_kernel_spmd(nc, in_maps, core_ids=list(range(NCORES)),
                               trace=trace)
    _CACHE["last_result"] = res

    out = np.zeros((B, S, D), dtype=np.float32)
    for c in range(NCORES):
        b = c // 4
        o = np.asarray(res.results[c]["outS"], dtype=np.float32)
        o = o.reshape(128, 16, 2048).transpose(1, 0, 2).reshape(S, D)
        out[b] += o
    out += bo[None, None, :]
    return out


# revision 7
# speedup vs baseline: 1.9455x; 1.0672x over previous
"""GQA attention block on 8 trn2 NeuronCores.

Sharding: core c = (batch b=c//4, kv-head-pair g=c%4). Each core owns kv heads
{2g, 2g+1} and their 8 query heads, with Wq/Wk/Wv column-sharded and Wo
row-sharded; host sums the 4 partial outputs per batch and adds bo.

Per-core schedule (v2 — PE-dense / ACT-dense pipeline):
  - scores row-packed: the two kv groups live on partitions 0-63 / 64-127 of
    K^T and Q^T, so each kt step issues two concurrent K=64 matmuls on
    disjoint PE row-groups into two PSUM banks (full 128-row utilization).
  - one [128,1024] exp over both banks per kt (amortizes ACT ramp overhead).
  - AV via lhsT=Vp with an appended ones column (M=65) giving the softmax
    denominator in row 64; normalization uses reciprocal_approx_fast +
    gpsimd broadcast, buffered 3-deep so it never stalls the kt pipeline.
  - RoPE: rotate_half built on DVE from the projection PSUM (no doubled
    weights), cos/sin combine writes bf16 Q^T/K^T pair tiles.
  - Q-proj / attention / O-proj share one pool scope; qpt and outT are
    double-buffered per s-quarter so quarter q+1's projections overlap
    quarter q's ACT-bound attention.
  - inputs host-restaged so every load/store is one dma_start with
    16KB/partition contiguous runs.
"""

import os
from contextlib import ExitStack

import numpy as np
import ml_dtypes

D = 2048
QH = 32
KVH = 8
HD = 64
B = 2
S = 2048
THETA = 1000000.0
P = 128
NCORES = 8

BF16 = ml_dtypes.bfloat16

_CACHE = {}


def _build_program():
    import concourse.bass as bass
    import concourse.tile as tile
    from concourse import bacc, mybir

    nc = bacc.Bacc(
        "TRN2",
        target_bir_lowering=False,
        debug=False,
        enable_asserts=False,
        num_devices=NCORES,
    )
    bf = mybir.dt.bfloat16
    f32 = mybir.dt.float32
    Exp = mybir.ActivationFunctionType.Exp

    # host-staged layouts: [p, t(=s quarter), o(=D/128 chunk), s] contiguous
    qS = nc.dram_tensor("qS", [P, 4 * 16 * 512], bf, kind="ExternalInput").ap()
    kS = nc.dram_tensor("kS", [P, 4 * 16 * 512], bf, kind="ExternalInput").ap()
    vS = nc.dram_tensor("vS", [P, 4 * 16 * 512], bf, kind="ExternalInput").ap()
    wqS = nc.dram_tensor("wqS", [P, 16 * 512], bf, kind="ExternalInput").ap()
    wkS = nc.dram_tensor("wkS", [P, 16 * 128], bf, kind="ExternalInput").ap()
    wvS = nc.dram_tensor("wvS", [P, 16 * 128], bf, kind="ExternalInput").ap()
    woS = nc.dram_tensor("woS", [P, 4 * 2048], bf, kind="ExternalInput").ap()
    cosr = nc.dram_tensor("cosr", [P, S], f32, kind="ExternalInput").ap()
    sinr = nc.dram_tensor("sinr", [P, S], f32, kind="ExternalInput").ap()
    outS = nc.dram_tensor("outS", [P, 16 * 2048], f32, kind="ExternalOutput").ap()

    q4 = qS.rearrange("p (t o s) -> p t o s", t=4, o=16, s=512)
    k4 = kS.rearrange("p (t o s) -> p t o s", t=4, o=16, s=512)
    v4 = vS.rearrange("p (t o s) -> p t o s", t=4, o=16, s=512)
    wq3 = wqS.rearrange("p (o m) -> p o m", o=16, m=512)
    wk3 = wkS.rearrange("p (o m) -> p o m", o=16, m=128)
    wv3 = wvS.rearrange("p (o m) -> p o m", o=16, m=128)
    wo3 = woS.rearrange("p (c d) -> p c d", c=4, d=2048)
    out3 = outS.rearrange("p (t d) -> p t d", t=16, d=2048)

    scale = 1.0 / float(np.sqrt(HD))

    with tile.TileContext(nc) as tc, ExitStack() as ctx:
        const = ctx.enter_context(tc.tile_pool(name="const", bufs=1))
        persist = ctx.enter_context(tc.tile_pool(name="persist", bufs=1))
        xin = ctx.enter_context(tc.tile_pool(name="xin", bufs=3))
        qptp = ctx.enter_context(tc.tile_pool(name="qptp", bufs=2))
        otp = ctx.enter_context(tc.tile_pool(name="otp", bufs=2))
        etp = ctx.enter_context(tc.tile_pool(name="etp", bufs=6))
        rtmp = ctx.enter_context(tc.tile_pool(name="rtmp", bufs=3))
        rcp = ctx.enter_context(tc.tile_pool(name="rcp", bufs=2))
        bcp = ctx.enter_context(tc.tile_pool(name="bcp", bufs=2))
        fout = ctx.enter_context(tc.tile_pool(name="fout", bufs=2))
        qpp = ctx.enter_context(tc.tile_pool(name="qpp", bufs=1, space="PSUM"))
        opp = ctx.enter_context(tc.tile_pool(name="opp", bufs=1, space="PSUM"))
        scp = ctx.enter_context(tc.tile_pool(name="scp", bufs=2, space="PSUM"))
        psop = ctx.enter_context(tc.tile_pool(name="psop", bufs=2, space="PSUM"))

        # ---- resident weights / tables ----
        wqt_sb = const.tile([P, 16, 512], bf, tag="wqt")
        nc.sync.dma_start(wqt_sb[:], wq3[:])
        wkt_sb = const.tile([P, 16, 128], bf, tag="wkt")
        nc.sync.dma_start(wkt_sb[:], wk3[:])
        wv_sb = const.tile([P, 16, 128], bf, tag="wv")
        nc.sync.dma_start(wv_sb[:], wv3[:])
        wo_sb = const.tile([P, 4, 2048], bf, tag="wo")
        nc.sync.dma_start(wo_sb[:], wo3[:])
        cos_sb = const.tile([P, S], f32, tag="cos")
        nc.sync.dma_start(cos_sb[:], cosr[:])
        sin_sb = const.tile([P, S], f32, tag="sin")
        nc.sync.dma_start(sin_sb[:], sinr[:])

        # ---- persistent intermediates ----
        kpt = persist.tile([P, S], bf, tag="kpt")      # rotated K^T, kv0|kv1 halves
        vp_t = []
        for vt in range(4):
            vv = persist.tile([P, 4, 130], bf, tag=f"vp{vt}")
            nc.vector.memset(vv[:, :, 64:65], 1.0)
            nc.vector.memset(vv[:, :, 129:130], 1.0)
            vp_t.append(vv)

        def rope(ps, dsts, tsl):
            """RoPE combine from psum tile ps [128,512] (2 head-blocks of 64)
            into dsts = [(dst_ap_for_rows_0_63), (dst_ap_for_rows_64_127)]."""
            rot = rtmp.tile([P, 512], f32, tag="rot")
            for b0 in (0, 64):
                nc.vector.tensor_scalar_mul(
                    rot[b0 : b0 + 32, :], ps[b0 + 32 : b0 + 64, :], -1.0
                )
                nc.vector.tensor_copy(
                    out=rot[b0 + 32 : b0 + 64, :], in_=ps[b0 : b0 + 32, :]
                )
            t1 = rtmp.tile([P, 512], f32, tag="t1")
            t2 = rtmp.tile([P, 512], f32, tag="t2")
            nc.vector.tensor_mul(out=t1[:], in0=ps[:], in1=cos_sb[:, tsl])
            nc.vector.tensor_mul(out=t2[:], in0=rot[:], in1=sin_sb[:, tsl])
            for i, dst in enumerate(dsts):
                rows = slice(i * 64, i * 64 + 64)
                nc.vector.tensor_add(out=dst, in0=t1[rows, :], in1=t2[rows, :])

        # ================= K projection + RoPE =================
        for t in range(4):
            kh = xin.tile([P, 16, 512], bf, tag="xin")
            nc.sync.dma_start(kh[:], k4[:, t])
            tsl = slice(t * 512, (t + 1) * 512)
            ps_full = scp.tile([P, 1024], f32, tag="sc", name="kproj")
            ps = ps_full[:, 0:512]
            for o in range(16):
                nc.tensor.matmul(
                    ps,
                    lhsT=wkt_sb[:, o, :],
                    rhs=kh[:, o, :],
                    start=(o == 0),
                    stop=(o == 15),
                )
            # kv0 -> kpt rows 0:64, kv1 -> rows 64:128 (same-row combine)
            rope(ps, [kpt[0:64, tsl], kpt[64:128, tsl]], tsl)

        # ---- Q projection + RoPE: pairs (j, j+4) on partition halves ----
        def qproj(t):
            tsl = slice(t * 512, (t + 1) * 512)
            qh = xin.tile([P, 16, 512], bf, tag="xin")
            nc.sync.dma_start(qh[:], q4[:, t])
            qpt = qptp.tile([P, 4, 512], bf, tag="qpt")
            for m in range(4):
                ps = qpp.tile([P, 512], f32, tag="qp", name="qproj")
                for o in range(16):
                    nc.tensor.matmul(
                        ps,
                        lhsT=wqt_sb[:, o, m * 128 : (m + 1) * 128],
                        rhs=qh[:, o, :],
                        start=(o == 0),
                        stop=(o == 15),
                    )
                dsts = []
                for h in (2 * m, 2 * m + 1):
                    base = 0 if h < 4 else 64
                    dsts.append(qpt[base : base + 64, h % 4, :])
                rope(ps, dsts, tsl)
            return qpt

        qpt_next = qproj(0)

        # ================= V projection =================
        for t in range(4):
            vh = xin.tile([P, 16, 512], bf, tag="xin")
            nc.sync.dma_start(vh[:], v4[:, t])
            for st in range(4):
                psv = psop.tile([P, 128], f32, tag="pso", name="psv")
                for o in range(16):
                    nc.tensor.matmul(
                        psv,
                        lhsT=vh[:, o, st * 128 : (st + 1) * 128],
                        rhs=wv_sb[:, o, :],
                        start=(o == 0),
                        stop=(o == 15),
                    )
                nc.vector.tensor_copy(out=vp_t[t][:, st, 0:64], in_=psv[:, 0:64])
                nc.vector.tensor_copy(out=vp_t[t][:, st, 65:129], in_=psv[:, 64:128])


        # ========== main loop: per s-quarter attn -> Qproj(t+1) -> outproj ==========
        for t in range(4):
            tsl = slice(t * 512, (t + 1) * 512)
            qpt = qpt_next

            # ---- attention: 4 head-pairs, row-packed scores ----
            outT = otp.tile([P, 4, 512], bf, tag="outT")
            for j in range(4):
                pso0 = psop.tile([65, 512], f32, tag="pso", name="pso0")
                pso1 = psop.tile([65, 512], f32, tag="pso", name="pso1")
                for kt in range(16):
                    ksl = slice(kt * 128, (kt + 1) * 128)
                    sc = scp.tile([P, 1024], f32, tag="sc", name="sc")
                    nc.tensor.matmul(
                        sc[:, 0:512],
                        lhsT=kpt[0:64, ksl],
                        rhs=qpt[0:64, j, :],
                        start=True,
                        stop=True,
                    )
                    nc.tensor.matmul(
                        sc[:, 512:1024],
                        lhsT=kpt[64:128, ksl],
                        rhs=qpt[64:128, j, :],
                        start=True,
                        stop=True,
                    )
                    et = etp.tile([P, 1024], bf, tag="et")
                    nc.scalar.activation(out=et[:], in_=sc[:], func=Exp, scale=scale)
                    vps = vp_t[kt // 4]
                    nc.tensor.matmul(
                        pso0,
                        lhsT=vps[:, kt % 4, 0:65],
                        rhs=et[:, 0:512],
                        start=(kt == 0),
                        stop=(kt == 15),
                    )
                    nc.tensor.matmul(
                        pso1,
                        lhsT=vps[:, kt % 4, 65:130],
                        rhs=et[:, 512:1024],
                        start=(kt == 0),
                        stop=(kt == 15),
                    )
                for pp, h in ((pso0, j), (pso1, j + 4)):
                    den = rcp.tile([1, 512], f32, tag="den")
                    nc.vector.tensor_copy(out=den[:], in_=pp[64:65, :])
                    rc = rcp.tile([1, 512], f32, tag="rc")
                    nc.vector.reciprocal_approx_fast(out=rc[:], in_=den[:])
                    bc = bcp.tile([64, 512], f32, tag="bc")
                    nc.gpsimd.partition_broadcast(bc[:], rc[:])
                    hp = (h % 2) * 64
                    nc.vector.tensor_mul(
                        out=outT[hp : hp + 64, h // 2, :], in0=pp[0:64, :], in1=bc[:]
                    )

            if t < 3:
                qpt_next = qproj(t + 1)

            # ---- output projection for this quarter's 4 s-tiles ----
            for qi in range(4):
                qt = t * 4 + qi
                fo = fout.tile([P, 2048], f32, tag="fo")
                for dn in range(4):
                    psf = opp.tile([P, 512], f32, tag="op", name="psf")
                    for cc in range(4):
                        nc.tensor.matmul(
                            psf,
                            lhsT=outT[:, cc, qi * 128 : (qi + 1) * 128],
                            rhs=wo_sb[:, cc, dn * 512 : (dn + 1) * 512],
                            start=(cc == 0),
                            stop=(cc == 3),
                        )
                    nc.vector.tensor_copy(
                        out=fo[:, dn * 512 : (dn + 1) * 512], in_=psf[:]
                    )
                nc.sync.dma_start(out3[:, qt], fo[:])

    nc.finalize()
    return nc


def _host_inputs(q, k, v, Wq, Wk, Wv, Wo):
    """Build the 8 per-core input dicts (host-restaged layouts)."""
    inv_freq = 1.0 / (THETA ** (np.arange(0, HD, 2, dtype=np.float32) / HD))
    tt = np.arange(S, dtype=np.float32)
    freqs = np.einsum("i,j->ij", tt, inv_freq)
    emb = np.concatenate([freqs, freqs], axis=-1)  # [S, 64]
    cosT = np.ascontiguousarray(np.cos(emb).T, dtype=np.float32)  # [64, S]
    sinT = np.ascontiguousarray(np.sin(emb).T, dtype=np.float32)
    cos_rep = np.ascontiguousarray(np.concatenate([cosT, cosT], axis=0))  # [128,S]
    sin_rep = np.ascontiguousarray(np.concatenate([sinT, sinT], axis=0))

    def stage_x(x):  # x [S, D] -> [p, t, o, s] flattened [128, 32768]
        a = np.ascontiguousarray(x.T).astype(BF16)  # [D, S]
        a = a.reshape(16, 128, 4, 512).transpose(1, 2, 0, 3)
        return np.ascontiguousarray(a).reshape(128, -1)

    xq = [stage_x(q[b]) for b in range(B)]
    xk = [stage_x(k[b]) for b in range(B)]
    xv = [stage_x(v[b]) for b in range(B)]

    in_maps = []
    for c in range(NCORES):
        b, g = divmod(c, 4)
        qheads = [2 * g, 2 * g + 8, 2 * g + 16, 2 * g + 24,
                  2 * g + 1, 2 * g + 9, 2 * g + 17, 2 * g + 25]
        qcols = np.concatenate([np.arange(h * HD, (h + 1) * HD) for h in qheads])
        kvcols = np.arange(2 * g * HD, (2 * g + 2) * HD)

        def stage_w(w, ncol):  # [D, ncol] -> [p, o, m] flattened
            a = w.astype(BF16).reshape(16, 128, ncol).transpose(1, 0, 2)
            return np.ascontiguousarray(a).reshape(128, -1)

        wq_np = stage_w(np.ascontiguousarray(Wq[:, qcols]), 512)
        wk_np = stage_w(np.ascontiguousarray(Wk[:, kvcols]), 128)
        wv_np = stage_w(np.ascontiguousarray(Wv[:, kvcols]), 128)
        wo_c = np.ascontiguousarray(Wo[qcols, :]).astype(BF16)  # [512, 2048]
        wo_np = np.ascontiguousarray(
            wo_c.reshape(4, 128, 2048).transpose(1, 0, 2)
        ).reshape(128, -1)

        in_maps.append({
            "qS": xq[b], "kS": xk[b], "vS": xv[b],
            "wqS": wq_np, "wkS": wk_np, "wvS": wv_np, "woS": wo_np,
            "cosr": cos_rep, "sinr": sin_rep,
        })
    return in_maps


def kernel(q, k, v, attn_mask, Wq, Wk, Wv, Wo, bo):
    from concourse.bass_utils import run_bass_kernel_spmd

    q = np.asarray(q, dtype=np.float32)
    k = np.asarray(k, dtype=np.float32)
    v = np.asarray(v, dtype=np.float32)
    Wq = np.asarray(Wq, dtype=np.float32)
    Wk = np.asarray(Wk, dtype=np.float32)
    Wv = np.asarray(Wv, dtype=np.float32)
    Wo = np.asarray(Wo, dtype=np.float32)
    bo = np.asarray(bo, dtype=np.float32)

    if "nc" not in _CACHE:
        _CACHE["nc"] = _build_program()
    nc = _CACHE["nc"]

    in_maps = _host_inputs(q, k, v, Wq, Wk, Wv, Wo)
    trace = bool(int(os.environ.get("KERNEL_TRACE", "0")))
    res = run_bass_kernel_spmd(nc, in_maps, core_ids=list(range(NCORES)),
                               trace=trace)
    _CACHE["last_result"] = res

    out = np.zeros((B, S, D), dtype=np.float32)
    for c in range(NCORES):
        b = c // 4
        o = np.asarray(res.results[c]["outS"], dtype=np.float32)
        o = o.reshape(128, 16, 2048).transpose(1, 0, 2).reshape(S, D)
        out[b] += o
    out += bo[None, None, :]
    return out
